# revision 39
# baseline (speedup 1.0000x reference)
"""DecoupledBottleneckAttention on 8 trn2 NeuronCores.

Sharding: core c -> batch b=c//4, head-group g=c%4 (4 heads/core).
Each core computes q/k/v projections for its heads, causal attention,
and a partial out-projection; the host sums the 4 partials per batch.

All matmul operands are bf16 (fp32 PSUM accumulation), which enables
FastWeightLoad (fp32 weights pay a full 128-cycle LDWEIGHTS per matmul),
halves DMA traffic, and doubles DVE throughput. x is staged transposed
[D, S] in SBUF once and both q/k and v projections read it from SBUF.
Scores are computed transposed (ST[k, q]) so softmax denominators come
from a ones-matmul and attn@V needs no transposes. exp() skips the
max-subtraction: logits are bounded (~|6|) by the fixed input scale.
Softmax reciprocals are batched [4, 512] per q-chunk (a lone [1, 512]
DVE reciprocal costs ~4us).
"""

import json
from contextlib import ExitStack

import numpy as np

import jax
import concourse.bass as bass
import concourse.mybir as mybir
from concourse.tile import TileContext
from concourse import bass2jax
from concourse.bass2jax import Mesh, PartitionSpec, shard_map, partition_id_tensor

F32 = mybir.dt.float32
F32R = mybir.dt.float32r
BF16 = mybir.dt.bfloat16
NP_BF16 = mybir.dt.np(BF16)

B, S, D = 2, 2048, 2048
H = 16
HPC = 4  # heads per core
N_CORES = 8
DH = 128  # per-head q/k/v dim (64 sem + 64 geo; v 128)
ROPE_BASE = 10000.0
SCALE = 1.0 / np.sqrt(128.0)

NSC = S // 512  # 4 s-chunks of 512
NDT = D // 128  # 16 contraction tiles
NST = S // 128  # 16 s-tiles of 128


def _split_multi_waits(bir: dict) -> dict:
    """walrus here rejects >1 sync waits per instruction; split extras
    into single-wait Drains inserted just before, on the same engine."""
    for fn in bir.get("functions", []):
        for blk in fn.get("blocks", []):
            new_insts = []
            for ins in blk.get("instructions", []):
                si = ins.get("sync_info") or {}
                waits = si.get("on_wait") or []
                if len(waits) > 1:
                    for i, w in enumerate(waits[:-1]):
                        new_insts.append(
                            {
                                "debug": ins.get("debug", 0),
                                "engine": ins["engine"],
                                "ins": [],
                                "name": f"{ins['name']}-w{i}",
                                "opcode": "Drain",
                                "outs": [],
                                "sync_info": {"on_update": [], "on_wait": [w]},
                            }
                        )
                    si["on_wait"] = [waits[-1]]
                new_insts.append(ins)
            blk["instructions"] = new_insts
    return bir


class _PatchedBass(bass.Bass):
    def to_json_bytes(self) -> bytes:
        return json.dumps(_split_multi_waits(json.loads(super().to_json_bytes()))).encode()


def _build():
    nc = _PatchedBass("TRN2", target_bir_lowering=False, debug=False, num_devices=N_CORES)

    xT_d = nc.dram_tensor("xT", [D, S], BF16, kind="ExternalInput")
    wqk_d = nc.dram_tensor("wqk", [D, 8 * 128], BF16, kind="ExternalInput")
    wv_d = nc.dram_tensor("wv", [D, HPC * DH], BF16, kind="ExternalInput")
    wo_d = nc.dram_tensor("wo", [HPC * DH, D], BF16, kind="ExternalInput")
    # rows 64:128 hold cos (cols 0:S) and sin (cols S:2S), replicated on
    # both 32-row geo half-ranges; rows 0:64 unused.
    cs_d = nc.dram_tensor("cs", [128, 2 * S], BF16, kind="ExternalInput")
    mask_d = nc.dram_tensor("mask", [128, 4 * 512], BF16, kind="ExternalInput")
    ones16_d = nc.dram_tensor("ones16", [128, 128], BF16, kind="ExternalInput")
    ones32_d = nc.dram_tensor("ones32", [128, 128], F32, kind="ExternalInput")
    yp_d = nc.dram_tensor("yp", [S, D], F32, kind="ExternalOutput")

    with TileContext(nc) as tc, ExitStack() as ctx, \
         nc.allow_low_precision(reason="bf16 matmul operands; fp32 accumulate"):
        pers = ctx.enter_context(tc.tile_pool(name="pers", bufs=1))
        # x kept resident, transposed, in [128, 512] chunks (sc-major) so
        # compute can start as soon as the first contraction tile lands.
        xt_sb = [pers.tile([128, 512], BF16, name=f"xt{t}", tag=f"xt{t}")
                 for t in range(NSC * NDT)]  # index sc * NDT + dt

        def xt(sc, dt):
            return xt_sb[sc * NDT + dt]

        # qkT[0..3] = per-head qT [128 dims, S]; qkT[4..7] = kT
        qkT = [pers.tile([128, S], BF16, name=f"qkT{i}", tag=f"qkT{i}") for i in range(8)]
        v_sb = [pers.tile([128, HPC * DH], BF16, name=f"v{st}", tag=f"v{st}")
                for st in range(NST)]
        outT = [pers.tile([128, S], BF16, name=f"outT{j}", tag=f"outT{j}")
                for j in range(HPC)]
        cs_sb = pers.tile([128, 2 * S], BF16, name="cs_sb", tag="cs_sb")
        mask_sb = pers.tile([128, 4 * 512], BF16, name="mask_sb", tag="mask_sb")
        ones16_sb = pers.tile([128, 128], BF16, name="ones16_sb", tag="ones16_sb")
        ones32_sb = pers.tile([128, 128], F32R, name="ones32_sb", tag="ones32_sb")

        # ---------------- Phases A+B: projections ----------------
        # psAB is shared by phases A and B (same tags/shapes) so B's first
        # accumulations recycle A's oldest slots instead of waiting for the
        # whole A pipeline to drain.
        with tc.tile_pool(name="psAB", bufs=2, space="PSUM") as psAB:
            # ----- Phase A: q/k projections + RoPE -----
            with tc.tile_pool(name="wqk", bufs=1) as wqk_pool, \
                 tc.tile_pool(name="ropeT", bufs=4) as ropeT:
                wqk_sb = [wqk_pool.tile([128, 8 * 128], BF16, name=f"wqk{dt}", tag=f"wqk{dt}")
                          for dt in range(NDT)]
                # load order: interleave wqk[dt] with xT chunk (sc=0, dt) so the
                # first accumulation chain is fed after ~0.5 MB instead of 14 MB;
                # remaining xT chunks and later-phase constants stream under
                # compute.
                for dt in range(NDT):
                    nc.sync.dma_start(out=wqk_sb[dt], in_=wqk_d[dt * 128:(dt + 1) * 128, :])
                    nc.sync.dma_start(out=xt(0, dt),
                                      in_=xT_d[dt * 128:(dt + 1) * 128, 0:512])
                nc.sync.dma_start(out=cs_sb, in_=cs_d[:, :])
                for sc in range(1, NSC):
                    for dt in range(NDT):
                        nc.sync.dma_start(
                            out=xt(sc, dt),
                            in_=xT_d[dt * 128:(dt + 1) * 128, sc * 512:(sc + 1) * 512])
                nc.sync.dma_start(out=mask_sb, in_=mask_d[:, :])
                nc.sync.dma_start(out=ones16_sb, in_=ones16_d[:, :])
                nc.sync.dma_start(out=ones32_sb, in_=ones32_d[:, :].bitcast(F32R))
                A, Bm = slice(64, 96), slice(96, 128)
                for sc in range(NSC):
                    cols = slice(sc * 512, (sc + 1) * 512)
                    csc = cs_sb[:, sc * 512:(sc + 1) * 512]       # cos, rows 64:128
                    sns = cs_sb[:, S + sc * 512:S + (sc + 1) * 512]  # sin, rows 64:128
                    for grp in range(2):
                        ps = [psAB.tile([128, 512], F32, name=f"psA{ob}", tag=f"psA{ob}")
                              for ob in range(4)]
                        for dt in range(NDT):
                            for ob in range(4):
                                blk = grp * 4 + ob
                                nc.tensor.matmul(
                                    ps[ob],
                                    lhsT=(wqk_sb[dt][:, blk * 128:(blk + 1) * 128]),
                                    rhs=(xt(sc, dt)),
                                    start=(dt == 0),
                                    stop=(dt == NDT - 1),
                                )
                        for ob in range(4):
                            blk = grp * 4 + ob
                            dst = qkT[blk]
                            # sem rows: plain copy
                            nc.scalar.activation(dst[0:64, cols], ps[ob][0:64, :],
                                                 mybir.ActivationFunctionType.Copy)
                            # geo rows: rotate-half RoPE. DVE operands must share a
                            # partition range and DMA cannot read PSUM, so stage the
                            # geo rows in SBUF, swap halves via SBUF->SBUF DMA, then
                            # all multiplies are partition-aligned.
                            stage = ropeT.tile([128, 512], BF16, name="ropest", tag="ropest")
                            sw = ropeT.tile([128, 512], BF16, name="ropesw", tag="ropesw")
                            tp = ropeT.tile([128, 512], BF16, name="ropetp", tag="ropetp")
                            nc.scalar.activation(stage[64:128, :], ps[ob][64:128, :],
                                                 mybir.ActivationFunctionType.Copy)
                            # scalar-engine DMA ring: the sync ring is busy
                            # streaming bulk input loads and would queue these
                            # 32 KB swaps behind megabytes of transfers.
                            nc.scalar.dma_start(out=sw[A, :], in_=stage[Bm, :])  # x2 -> A
                            nc.scalar.dma_start(out=sw[Bm, :], in_=stage[A, :])  # x1 -> B
                            o1, o2 = dst[A, cols], dst[Bm, cols]
                            nc.vector.tensor_mul(o1, stage[A, :], csc[A, :])     # x1*cos
                            nc.vector.tensor_mul(tp[A, :], sw[A, :], sns[A, :])  # x2*sin
                            nc.vector.tensor_sub(o1, o1, tp[A, :])
                            nc.vector.tensor_mul(o2, stage[Bm, :], csc[Bm, :])   # x2*cos
                            nc.vector.tensor_mul(tp[Bm, :], sw[Bm, :], sns[Bm, :])  # x1*sin
                            nc.vector.tensor_add(o2, o2, tp[Bm, :])

        # ------------- Phases B+C: v projection interleaved into attention ---
        # Phase C's PE work sits well below its ACT/DVE floor (exp + mask /
        # accumulate), so PE would idle and HAM would re-throttle the clock.
        # Interleaving phase B's PE-dense matmuls into phase C's emission
        # keeps the tensor engine saturated through the whole window.
        with tc.tile_pool(name="wv", bufs=1) as wv_pool, \
             tc.tile_pool(name="attn", bufs=5) as attn_pool, \
             tc.tile_pool(name="pacc", bufs=2) as pacc_pool, \
             tc.tile_pool(name="lrec", bufs=3) as lrec_pool, \
             tc.tile_pool(name="psB", bufs=1, space="PSUM") as psB, \
             tc.tile_pool(name="psST", bufs=3, space="PSUM") as psST, \
             tc.tile_pool(name="psOut", bufs=2, space="PSUM") as psOut, \
             tc.tile_pool(name="psL", bufs=1, space="PSUM") as psL:
            wv_sb = [wv_pool.tile([128, HPC * DH], BF16, name=f"wv{dt}", tag=f"wv{dt}")
                     for dt in range(NDT)]
            for dt in range(NDT):
                nc.sync.dma_start(out=wv_sb[dt], in_=wv_d[dt * 128:(dt + 1) * 128, :])

            def emit_b_chunk(sc):
                """Generator: phase-B matmuls for seq chunk sc in 8 pieces
                (2 waves of 2 s-tiles, 4 dt-quarters each) so they can be
                sprinkled through phase C's emission."""
                for w in range(2):
                    psv = [psB.tile([128, HPC * DH], F32, name=f"psB{i}", tag=f"psB{i}")
                           for i in range(2)]
                    for dq in range(4):
                        for dt in range(dq * 4, (dq + 1) * 4):
                            for i in range(2):
                                st = w * 2 + i
                                nc.tensor.matmul(
                                    psv[i],
                                    lhsT=(xt(sc, dt)[:, st * 128:(st + 1) * 128]),
                                    rhs=(wv_sb[dt]),
                                    start=(dt == 0),
                                    stop=(dt == NDT - 1),
                                )
                        yield
                    for i in range(2):
                        nc.scalar.activation(v_sb[sc * 4 + w * 2 + i], psv[i],
                                             mybir.ActivationFunctionType.Copy)

            def drain(gen, n=None):
                if gen is None:
                    return None
                try:
                    if n is None:
                        while True:
                            next(gen)
                    else:
                        for _ in range(n):
                            next(gen)
                except StopIteration:
                    return None
                return gen

            # B(0) must complete before C(0) consumes its v tiles
            drain(emit_b_chunk(0))

            # Normalization for head (qc, j) is deferred one head: the
            # broadcast matmul rp sits in PE's in-order stream and deferring
            # gives the reciprocal chain time to finish, so PE never stalls.
            pending = []

            def flush_norm():
                r_sb, outp, j, qcols = pending.pop(0)
                rp = psST.tile([128, 512], F32, name="st_ps", tag="st_ps")
                nc.tensor.matmul(rp, lhsT=(ones32_sb[0:1, :]),
                                 rhs=(r_sb), start=True, stop=True)
                # DVE may read only one PSUM operand per op: stage rp first
                rbc = lrec_pool.tile([128, 512], F32, name="rbc", tag="rbc")
                nc.vector.tensor_copy(rbc, rp)
                nc.vector.tensor_mul(outT[j][:, qcols], outp, rbc)

            for qc in range(NSC):
                qcols = slice(qc * 512, (qc + 1) * 512)
                kmax = qc * 4 + 4
                bgen = emit_b_chunk(qc + 1) if qc + 1 < NSC else None
                for j in range(HPC):
                    # two pieces of the next B chunk per head keeps PE fed
                    # while ACT/DVE work through exp/mask/accumulate
                    bgen = drain(bgen, 2)
                    outp = psOut.tile([128, 512], F32, name="outp", tag="outp")
                    # running sum of the probability blocks on DVE; one
                    # ones-matmul per (j, qc) then turns it into the softmax
                    # denominator
                    pacc = pacc_pool.tile([128, 512], BF16, name="pacc", tag="pacc")

                    def emit_av(kj, p_sb):
                        nc.tensor.matmul(
                            outp,
                            lhsT=(v_sb[kj][:, j * DH:(j + 1) * DH]),
                            rhs=(p_sb),
                            start=(kj == 0), stop=(kj == kmax - 1),
                        )

                    pend_av = []
                    for kj in range(kmax):
                        st_ps = psST.tile([128, 512], F32, name="st_ps", tag="st_ps")
                        nc.tensor.matmul(
                            st_ps,
                            lhsT=(qkT[4 + j][:, kj * 128:(kj + 1) * 128]),
                            rhs=(qkT[j][:, qcols]),
                            start=True, stop=True,
                        )
                        p_sb = attn_pool.tile([128, 512], BF16, name="p_sb", tag="p_sb")
                        nc.scalar.activation(p_sb, st_ps,
                                             mybir.ActivationFunctionType.Exp)
                        dj = kj - qc * 4
                        if dj >= 0:
                            nc.vector.tensor_mul(
                                p_sb, p_sb, mask_sb[:, dj * 512:(dj + 1) * 512])
                        if kj == 0:
                            nc.vector.tensor_copy(pacc, p_sb)
                        else:
                            nc.vector.tensor_add(pacc, pacc, p_sb)
                        pend_av.append((kj, p_sb))
                        if len(pend_av) > 3:
                            emit_av(*pend_av.pop(0))
                    while pend_av:
                        emit_av(*pend_av.pop(0))
                    lp = psL.tile([1, 512], F32, name="lp", tag="lp")
                    nc.tensor.matmul(lp, lhsT=(ones16_sb[:, 0:1]), rhs=(pacc),
                                     start=True, stop=True)
                    l_sb = lrec_pool.tile([1, 512], F32, name="l_sb", tag="l_sb")
                    nc.scalar.activation(l_sb, lp,
                                         mybir.ActivationFunctionType.Copy)
                    # DVE reciprocal is an iterative 8-cycle/element op along
                    # the free dim, so [1,512] costs ~3.3us. Spreading the 512
                    # values over 128 partitions via DMA makes it ~4 elements
                    # per lane (~0.2us); a second DMA packs them back.
                    lw = lrec_pool.tile([128, 4], F32, name="lw", tag="lw")
                    nc.scalar.dma_start(out=lw, in_=l_sb)
                    rw = lrec_pool.tile([128, 4], F32R, name="rw", tag="rw")
                    nc.vector.reciprocal(rw, lw)
                    r_sb = lrec_pool.tile([1, 512], F32R, name="r_sb", tag="r_sb")
                    nc.scalar.dma_start(out=r_sb, in_=rw)
                    pending.append((r_sb, outp, j, qcols))
                    if len(pending) > 1:
                        flush_norm()
                drain(bgen)
            while pending:
                flush_norm()

        # ------------- Phase D: out-projection ------------------
        with tc.tile_pool(name="wo", bufs=1) as wo_pool, \
             tc.tile_pool(name="ysb", bufs=2) as y_pool, \
             tc.tile_pool(name="psD", bufs=2, space="PSUM") as psD:
            wo_sb = [wo_pool.tile([128, D], BF16, name=f"wo{j}", tag=f"wo{j}")
                     for j in range(HPC)]
            for j in range(HPC):
                nc.sync.dma_start(out=wo_sb[j], in_=wo_d[j * 128:(j + 1) * 128, :])
            for st in range(NST):
                y_row = y_pool.tile([128, D], F32, name="y_row", tag="y_row")
                # j-major so the stationary operand (outT[j] slice) is reused
                # across the 4 output chunks (1 LDWEIGHTS per 4 matmuls).
                yp_ps = [psD.tile([128, 512], F32, name=f"yp_ps{mc}", tag=f"yp_ps{mc}")
                         for mc in range(NSC)]
                for j in range(HPC):
                    for mc in range(NSC):
                        nc.tensor.matmul(
                            yp_ps[mc],
                            lhsT=(outT[j][:, st * 128:(st + 1) * 128]),
                            rhs=(wo_sb[j][:, mc * 512:(mc + 1) * 512]),
                            start=(j == 0), stop=(j == HPC - 1),
                        )
                for mc in range(NSC):
                    nc.vector.tensor_copy(y_row[:, mc * 512:(mc + 1) * 512], yp_ps[mc])
                nc.scalar.dma_start(out=yp_d[st * 128:(st + 1) * 128, :], in_=y_row)
    return nc


class SpmdRunner:
    """Donation-free SPMD runner: the kernel writes every output element,
    so PJRT-allocated (uninitialized) output buffers are fine and no
    zero-buffer round trip is needed per call."""

    def __init__(self, nc, n_cores: int):
        bass2jax.install_neuronx_cc_hook()
        self.nc = nc
        self.n_cores = n_cores
        partition_name = nc.partition_id_tensor.name if nc.partition_id_tensor else None

        in_names, out_names, out_avals = [], [], []
        for alloc in nc.m.functions[0].allocations:
            if not isinstance(alloc, mybir.MemoryLocationSet):
                continue
            name = alloc.memorylocations[0].name
            if alloc.kind == "ExternalInput":
                if name != partition_name:
                    in_names.append(name)
            elif alloc.kind == "ExternalOutput":
                out_names.append(name)
                shape = tuple(alloc.tensor_shape)
                dtype = mybir.dt.np(alloc.dtype)
                out_avals.append(jax.core.ShapedArray(shape, dtype))
        self.in_names = in_names
        self.out_names = out_names
        self.out_avals = out_avals

        all_in_names = list(in_names)
        if partition_name is not None:
            all_in_names.append(partition_name)

        def _body(*args):
            operands = list(args)
            if partition_name is not None:
                operands.append(partition_id_tensor())
            outs = bass2jax._bass_exec_p.bind(
                *operands,
                out_avals=tuple(out_avals),
                in_names=tuple(all_in_names),
                out_names=tuple(out_names),
                lowering_input_output_aliases=(),
                sim_require_finite=True,
                sim_require_nnan=True,
                nc=nc,
            )
            return tuple(outs)

        devices = jax.devices()[:n_cores]
        self.mesh = Mesh(np.asarray(devices), ("core",))
        in_specs = (PartitionSpec("core"),) * len(in_names)
        out_specs = (PartitionSpec("core"),) * len(out_names)
        self.jitted = jax.jit(
            shard_map(_body, mesh=self.mesh, in_specs=in_specs,
                      out_specs=out_specs, check_rep=False),
            keep_unused=True,
        )
        self.sharding = jax.sharding.NamedSharding(self.mesh, PartitionSpec("core"))

    def stage(self, in_maps):
        assert len(in_maps) == self.n_cores
        arrs = [
            np.concatenate([np.asarray(in_maps[c][n]) for c in range(self.n_cores)], axis=0)
            for n in self.in_names
        ]
        staged = [jax.device_put(a, self.sharding) for a in arrs]
        jax.block_until_ready(staged)
        return staged

    def run_staged(self, staged):
        outs = self.jitted(*staged)
        jax.block_until_ready(outs)
        return outs

    def __call__(self, in_maps):
        staged = self.stage(in_maps)
        outs = self.run_staged(staged)
        res = []
        for c in range(self.n_cores):
            res.append({
                name: np.asarray(outs[i]).reshape(self.n_cores, *self.out_avals[i].shape)[c]
                for i, name in enumerate(self.out_names)
            })
        return res


_NC_CACHE: dict = {}


def _get_runner():
    if "runner" not in _NC_CACHE:
        _NC_CACHE["runner"] = SpmdRunner(_build(), N_CORES)
    return _NC_CACHE["runner"]


def _host_inputs(x, Wq_sem, Wk_sem, Wq_geo, Wk_geo, Wv, Wo):
    # RoPE tables
    inv_freq = 1.0 / (ROPE_BASE ** (np.arange(0, 64, 2, dtype=np.float32) / 64.0))
    t = np.arange(S, dtype=np.float32)
    freqs = np.outer(t, inv_freq)  # [S, 32]
    cosT = np.cos(freqs).T.astype(np.float32)  # [32, S]
    sinT = np.sin(freqs).T.astype(np.float32)
    cs = np.zeros((128, 2 * S), np.float32)
    cs[64:96, :S] = cosT
    cs[96:128, :S] = cosT
    cs[64:96, S:] = sinT
    cs[96:128, S:] = sinT
    cs = cs.astype(NP_BF16)

    # causal mask variants: mask[kl, dj*512 + ql] = ql >= dj*128 + kl
    ql = np.arange(512)
    kl = np.arange(128)
    mask = np.zeros((128, 4 * 512), np.float32)
    for dj in range(4):
        mask[:, dj * 512:(dj + 1) * 512] = (ql[None, :] >= dj * 128 + kl[:, None])
    mask = mask.astype(NP_BF16)

    ones16 = np.ones((128, 128), NP_BF16)
    ones32 = np.ones((128, 128), np.float32)

    in_maps = []
    for c in range(N_CORES):
        b, g = divmod(c, 4)
        blocks_q, blocks_k = [], []
        for j in range(HPC):
            h = g * HPC + j
            r64 = slice(h * 64, (h + 1) * 64)
            blocks_q.append(np.concatenate([Wq_sem[r64], Wq_geo[r64]], axis=0) * SCALE)
            blocks_k.append(np.concatenate([Wk_sem[r64], Wk_geo[r64]], axis=0))
        wqk = np.ascontiguousarray(np.concatenate(blocks_q + blocks_k, axis=0).T)
        hv = slice(g * HPC * DH, (g + 1) * HPC * DH)
        wv = np.ascontiguousarray(Wv[hv].T)
        wo = np.ascontiguousarray(Wo[:, hv].T)
        xT = np.ascontiguousarray(x[b].T)
        in_maps.append({
            "xT": xT.astype(NP_BF16),
            "wqk": wqk.astype(NP_BF16),
            "wv": wv.astype(NP_BF16),
            "wo": wo.astype(NP_BF16),
            "cs": cs,
            "mask": mask,
            "ones16": ones16,
            "ones32": ones32,
        })
    return in_maps


def kernel(x, Wq_sem, Wk_sem, Wq_geo, Wk_geo, Wv, Wo):
    in_maps = _host_inputs(np.asarray(x), np.asarray(Wq_sem), np.asarray(Wk_sem),
                           np.asarray(Wq_geo), np.asarray(Wk_geo),
                           np.asarray(Wv), np.asarray(Wo))
    res = _get_runner()(in_maps)
    y = np.empty((B, S, D), np.float32)
    for b in range(B):
        y[b] = sum(res[b * 4 + g]["yp"] for g in range(4))
    return y


# revision 40
# speedup vs baseline: 1.0334x; 1.0334x over previous
"""DecoupledBottleneckAttention on 8 trn2 NeuronCores.

Sharding: core c -> batch b=c//4, head-group g=c%4 (4 heads/core).
Each core computes q/k/v projections for its heads, causal attention,
and a partial out-projection; the host sums the 4 partials per batch.

All matmul operands are bf16 (fp32 PSUM accumulation), which enables
FastWeightLoad (fp32 weights pay a full 128-cycle LDWEIGHTS per matmul),
halves DMA traffic, and doubles DVE throughput. x is staged transposed
[D, S] in SBUF once and both q/k and v projections read it from SBUF.
Scores are computed transposed (ST[k, q]) so softmax denominators come
from a ones-matmul and attn@V needs no transposes. exp() skips the
max-subtraction: logits are bounded (~|6|) by the fixed input scale.
Softmax reciprocals are batched [4, 512] per q-chunk (a lone [1, 512]
DVE reciprocal costs ~4us).
"""

import json
from contextlib import ExitStack

import numpy as np

import jax
import concourse.bass as bass
import concourse.mybir as mybir
from concourse.tile import TileContext
from concourse import bass2jax
from concourse.bass2jax import Mesh, PartitionSpec, shard_map, partition_id_tensor

F32 = mybir.dt.float32
F32R = mybir.dt.float32r
BF16 = mybir.dt.bfloat16
NP_BF16 = mybir.dt.np(BF16)

B, S, D = 2, 2048, 2048
H = 16
HPC = 4  # heads per core
N_CORES = 8
DH = 128  # per-head q/k/v dim (64 sem + 64 geo; v 128)
ROPE_BASE = 10000.0
SCALE = 1.0 / np.sqrt(128.0)

NSC = S // 512  # 4 s-chunks of 512
NDT = D // 128  # 16 contraction tiles
NST = S // 128  # 16 s-tiles of 128


def _split_multi_waits(bir: dict) -> dict:
    """walrus here rejects >1 sync waits per instruction; split extras
    into single-wait Drains inserted just before, on the same engine."""
    for fn in bir.get("functions", []):
        for blk in fn.get("blocks", []):
            new_insts = []
            for ins in blk.get("instructions", []):
                si = ins.get("sync_info") or {}
                waits = si.get("on_wait") or []
                if len(waits) > 1:
                    for i, w in enumerate(waits[:-1]):
                        new_insts.append(
                            {
                                "debug": ins.get("debug", 0),
                                "engine": ins["engine"],
                                "ins": [],
                                "name": f"{ins['name']}-w{i}",
                                "opcode": "Drain",
                                "outs": [],
                                "sync_info": {"on_update": [], "on_wait": [w]},
                            }
                        )
                    si["on_wait"] = [waits[-1]]
                new_insts.append(ins)
            blk["instructions"] = new_insts
    return bir


class _PatchedBass(bass.Bass):
    def to_json_bytes(self) -> bytes:
        return json.dumps(_split_multi_waits(json.loads(super().to_json_bytes()))).encode()


def _build():
    nc = _PatchedBass("TRN2", target_bir_lowering=False, debug=False, num_devices=N_CORES)

    xT_d = nc.dram_tensor("xT", [D, S], BF16, kind="ExternalInput")
    wqk_d = nc.dram_tensor("wqk", [D, 8 * 128], BF16, kind="ExternalInput")
    wv_d = nc.dram_tensor("wv", [D, HPC * DH], BF16, kind="ExternalInput")
    wo_d = nc.dram_tensor("wo", [HPC * DH, D], BF16, kind="ExternalInput")
    # rows 64:128 hold cos (cols 0:S) and sin (cols S:2S), replicated on
    # both 32-row geo half-ranges; rows 0:64 unused.
    cs_d = nc.dram_tensor("cs", [128, 2 * S], BF16, kind="ExternalInput")
    mask_d = nc.dram_tensor("mask", [128, 4 * 512], BF16, kind="ExternalInput")
    ones16_d = nc.dram_tensor("ones16", [128, 128], BF16, kind="ExternalInput")
    ones32_d = nc.dram_tensor("ones32", [128, 128], F32, kind="ExternalInput")
    yp_d = nc.dram_tensor("yp", [S, D], F32, kind="ExternalOutput")

    with TileContext(nc) as tc, ExitStack() as ctx, \
         nc.allow_low_precision(reason="bf16 matmul operands; fp32 accumulate"):
        pers = ctx.enter_context(tc.tile_pool(name="pers", bufs=1))
        # x kept resident, transposed, in [128, 512] chunks (sc-major) so
        # compute can start as soon as the first contraction tile lands.
        xt_sb = [pers.tile([128, 512], BF16, name=f"xt{t}", tag=f"xt{t}")
                 for t in range(NSC * NDT)]  # index sc * NDT + dt

        def xt(sc, dt):
            return xt_sb[sc * NDT + dt]

        # qkT[0..3] = per-head qT [128 dims, S]; qkT[4..7] = kT
        qkT = [pers.tile([128, S], BF16, name=f"qkT{i}", tag=f"qkT{i}") for i in range(8)]
        v_sb = [pers.tile([128, HPC * DH], BF16, name=f"v{st}", tag=f"v{st}")
                for st in range(NST)]
        outT = [pers.tile([128, S], BF16, name=f"outT{j}", tag=f"outT{j}")
                for j in range(HPC)]
        cs_sb = pers.tile([128, 2 * S], BF16, name="cs_sb", tag="cs_sb")
        mask_sb = pers.tile([128, 4 * 512], BF16, name="mask_sb", tag="mask_sb")
        ones16_sb = pers.tile([128, 128], BF16, name="ones16_sb", tag="ones16_sb")
        ones32_sb = pers.tile([128, 128], F32R, name="ones32_sb", tag="ones32_sb")

        # ---------------- Phases A+B: projections ----------------
        # psAB is shared by phases A and B (same tags/shapes) so B's first
        # accumulations recycle A's oldest slots instead of waiting for the
        # whole A pipeline to drain.
        with tc.tile_pool(name="psAB", bufs=2, space="PSUM") as psAB:
            # ----- Phase A: q/k projections + RoPE -----
            with tc.tile_pool(name="wqk", bufs=1) as wqk_pool, \
                 tc.tile_pool(name="ropeT", bufs=4) as ropeT:
                wqk_sb = [wqk_pool.tile([128, 8 * 128], BF16, name=f"wqk{dt}", tag=f"wqk{dt}")
                          for dt in range(NDT)]
                # load order: interleave wqk[dt] with xT chunk (sc=0, dt) so the
                # first accumulation chain is fed after ~0.5 MB instead of 14 MB;
                # remaining xT chunks and later-phase constants stream under
                # compute.
                for dt in range(NDT):
                    nc.sync.dma_start(out=wqk_sb[dt], in_=wqk_d[dt * 128:(dt + 1) * 128, :])
                    nc.sync.dma_start(out=xt(0, dt),
                                      in_=xT_d[dt * 128:(dt + 1) * 128, 0:512])
                nc.sync.dma_start(out=cs_sb, in_=cs_d[:, :])
                for sc in range(1, NSC):
                    for dt in range(NDT):
                        nc.sync.dma_start(
                            out=xt(sc, dt),
                            in_=xT_d[dt * 128:(dt + 1) * 128, sc * 512:(sc + 1) * 512])
                nc.sync.dma_start(out=mask_sb, in_=mask_d[:, :])
                nc.sync.dma_start(out=ones16_sb, in_=ones16_d[:, :])
                nc.sync.dma_start(out=ones32_sb, in_=ones32_d[:, :].bitcast(F32R))
                A, Bm = slice(64, 96), slice(96, 128)
                for sc in range(NSC):
                    cols = slice(sc * 512, (sc + 1) * 512)
                    csc = cs_sb[:, sc * 512:(sc + 1) * 512]       # cos, rows 64:128
                    sns = cs_sb[:, S + sc * 512:S + (sc + 1) * 512]  # sin, rows 64:128
                    for grp in range(2):
                        ps = [psAB.tile([128, 512], F32, name=f"psA{ob}", tag=f"psA{ob}")
                              for ob in range(4)]
                        for dt in range(NDT):
                            for ob in range(4):
                                blk = grp * 4 + ob
                                nc.tensor.matmul(
                                    ps[ob],
                                    lhsT=(wqk_sb[dt][:, blk * 128:(blk + 1) * 128]),
                                    rhs=(xt(sc, dt)),
                                    start=(dt == 0),
                                    stop=(dt == NDT - 1),
                                )
                        for ob in range(4):
                            blk = grp * 4 + ob
                            dst = qkT[blk]
                            # sem rows: plain copy
                            nc.scalar.activation(dst[0:64, cols], ps[ob][0:64, :],
                                                 mybir.ActivationFunctionType.Copy)
                            # geo rows: rotate-half RoPE. DVE operands must share a
                            # partition range and DMA cannot read PSUM, so stage the
                            # geo rows in SBUF, swap halves via SBUF->SBUF DMA, then
                            # all multiplies are partition-aligned.
                            stage = ropeT.tile([128, 512], BF16, name="ropest", tag="ropest")
                            sw = ropeT.tile([128, 512], BF16, name="ropesw", tag="ropesw")
                            tp = ropeT.tile([128, 512], BF16, name="ropetp", tag="ropetp")
                            nc.scalar.activation(stage[64:128, :], ps[ob][64:128, :],
                                                 mybir.ActivationFunctionType.Copy)
                            # scalar-engine DMA ring: the sync ring is busy
                            # streaming bulk input loads and would queue these
                            # 32 KB swaps behind megabytes of transfers.
                            nc.scalar.dma_start(out=sw[A, :], in_=stage[Bm, :])  # x2 -> A
                            nc.scalar.dma_start(out=sw[Bm, :], in_=stage[A, :])  # x1 -> B
                            o1, o2 = dst[A, cols], dst[Bm, cols]
                            nc.vector.tensor_mul(o1, stage[A, :], csc[A, :])     # x1*cos
                            nc.vector.tensor_mul(tp[A, :], sw[A, :], sns[A, :])  # x2*sin
                            nc.vector.tensor_sub(o1, o1, tp[A, :])
                            nc.vector.tensor_mul(o2, stage[Bm, :], csc[Bm, :])   # x2*cos
                            nc.vector.tensor_mul(tp[Bm, :], sw[Bm, :], sns[Bm, :])  # x1*sin
                            nc.vector.tensor_add(o2, o2, tp[Bm, :])

        # ------------- Phases B+C: v projection interleaved into attention ---
        # Phase C's PE work sits well below its ACT/DVE floor (exp + mask /
        # accumulate), so PE would idle and HAM would re-throttle the clock.
        # Interleaving phase B's PE-dense matmuls into phase C's emission
        # keeps the tensor engine saturated through the whole window.
        with tc.tile_pool(name="wv", bufs=1) as wv_pool, \
             tc.tile_pool(name="attn", bufs=5) as attn_pool, \
             tc.tile_pool(name="pacc", bufs=2) as pacc_pool, \
             tc.tile_pool(name="lrec", bufs=3) as lrec_pool, \
             tc.tile_pool(name="psB", bufs=1, space="PSUM") as psB, \
             tc.tile_pool(name="psST", bufs=3, space="PSUM") as psST, \
             tc.tile_pool(name="psOut", bufs=2, space="PSUM") as psOut, \
             tc.tile_pool(name="psL", bufs=1, space="PSUM") as psL:
            wv_sb = [wv_pool.tile([128, HPC * DH], BF16, name=f"wv{dt}", tag=f"wv{dt}")
                     for dt in range(NDT)]
            for dt in range(NDT):
                nc.sync.dma_start(out=wv_sb[dt], in_=wv_d[dt * 128:(dt + 1) * 128, :])

            def emit_b_chunk(sc):
                """Generator: phase-B matmuls for seq chunk sc in 8 pieces
                (2 waves of 2 s-tiles, 4 dt-quarters each) so they can be
                sprinkled through phase C's emission."""
                for w in range(2):
                    psv = [psB.tile([128, HPC * DH], F32, name=f"psB{i}", tag=f"psB{i}")
                           for i in range(2)]
                    for dq in range(4):
                        for dt in range(dq * 4, (dq + 1) * 4):
                            for i in range(2):
                                st = w * 2 + i
                                nc.tensor.matmul(
                                    psv[i],
                                    lhsT=(xt(sc, dt)[:, st * 128:(st + 1) * 128]),
                                    rhs=(wv_sb[dt]),
                                    start=(dt == 0),
                                    stop=(dt == NDT - 1),
                                )
                        yield
                    for i in range(2):
                        nc.scalar.activation(v_sb[sc * 4 + w * 2 + i], psv[i],
                                             mybir.ActivationFunctionType.Copy)

            def drain(gen, n=None):
                if gen is None:
                    return None
                try:
                    if n is None:
                        while True:
                            next(gen)
                    else:
                        for _ in range(n):
                            next(gen)
                except StopIteration:
                    return None
                return gen

            # B(0) must complete before C(0) consumes its v tiles
            drain(emit_b_chunk(0))

            # Normalization for head (qc, j) is deferred one head: the
            # broadcast matmul rp sits in PE's in-order stream and deferring
            # gives the reciprocal chain time to finish, so PE never stalls.
            pending = []

            def flush_norm():
                r_sb, outp, j, qcols = pending.pop(0)
                rp = psST.tile([128, 512], F32, name="st_ps", tag="st_ps")
                nc.tensor.matmul(rp, lhsT=(ones32_sb[0:1, :]),
                                 rhs=(r_sb), start=True, stop=True)
                # DVE may read only one PSUM operand per op: stage rp first
                rbc = lrec_pool.tile([128, 512], F32, name="rbc", tag="rbc")
                nc.vector.tensor_copy(rbc, rp)
                nc.vector.tensor_mul(outT[j][:, qcols], outp, rbc)

            for qc in range(NSC):
                qcols = slice(qc * 512, (qc + 1) * 512)
                kmax = qc * 4 + 4
                bgen = emit_b_chunk(qc + 1) if qc + 1 < NSC else None
                for j in range(HPC):
                    # two pieces of the next B chunk per head keeps PE fed
                    # while ACT/DVE work through exp/mask/accumulate
                    bgen = drain(bgen, 2)
                    outp = psOut.tile([128, 512], F32, name="outp", tag="outp")
                    # running sum of the probability blocks on DVE; one
                    # ones-matmul per (j, qc) then turns it into the softmax
                    # denominator
                    pacc = pacc_pool.tile([128, 512], BF16, name="pacc", tag="pacc")

                    def emit_av(kj, p_sb):
                        nc.tensor.matmul(
                            outp,
                            lhsT=(v_sb[kj][:, j * DH:(j + 1) * DH]),
                            rhs=(p_sb),
                            start=(kj == 0), stop=(kj == kmax - 1),
                        )

                    pend_av = []
                    for kj in range(kmax):
                        st_ps = psST.tile([128, 512], F32, name="st_ps", tag="st_ps")
                        nc.tensor.matmul(
                            st_ps,
                            lhsT=(qkT[4 + j][:, kj * 128:(kj + 1) * 128]),
                            rhs=(qkT[j][:, qcols]),
                            start=True, stop=True,
                        )
                        p_sb = attn_pool.tile([128, 512], BF16, name="p_sb", tag="p_sb")
                        nc.scalar.activation(p_sb, st_ps,
                                             mybir.ActivationFunctionType.Exp)
                        dj = kj - qc * 4
                        if dj >= 0:
                            nc.vector.tensor_mul(
                                p_sb, p_sb, mask_sb[:, dj * 512:(dj + 1) * 512])
                        if kj == 0:
                            nc.vector.tensor_copy(pacc, p_sb)
                        else:
                            nc.vector.tensor_add(pacc, pacc, p_sb)
                        pend_av.append((kj, p_sb))
                        if len(pend_av) > 3:
                            emit_av(*pend_av.pop(0))
                    while pend_av:
                        emit_av(*pend_av.pop(0))
                    lp = psL.tile([1, 512], F32, name="lp", tag="lp")
                    nc.tensor.matmul(lp, lhsT=(ones16_sb[:, 0:1]), rhs=(pacc),
                                     start=True, stop=True)
                    l_sb = lrec_pool.tile([1, 512], F32, name="l_sb", tag="l_sb")
                    nc.scalar.activation(l_sb, lp,
                                         mybir.ActivationFunctionType.Copy)
                    # DVE reciprocal is an iterative 8-cycle/element op along
                    # the free dim, so [1,512] costs ~3.3us. Spreading the 512
                    # values over 128 partitions via DMA makes it ~4 elements
                    # per lane (~0.2us); a second DMA packs them back.
                    lw = lrec_pool.tile([128, 4], F32, name="lw", tag="lw")
                    nc.gpsimd.dma_start(out=lw, in_=l_sb)
                    rw = lrec_pool.tile([128, 4], F32R, name="rw", tag="rw")
                    nc.vector.reciprocal(rw, lw)
                    r_sb = lrec_pool.tile([1, 512], F32R, name="r_sb", tag="r_sb")
                    nc.gpsimd.dma_start(out=r_sb, in_=rw)
                    pending.append((r_sb, outp, j, qcols))
                    if len(pending) > 1:
                        flush_norm()
                drain(bgen)
            while pending:
                flush_norm()

        # ------------- Phase D: out-projection ------------------
        with tc.tile_pool(name="wo", bufs=1) as wo_pool, \
             tc.tile_pool(name="ysb", bufs=2) as y_pool, \
             tc.tile_pool(name="psD", bufs=2, space="PSUM") as psD:
            wo_sb = [wo_pool.tile([128, D], BF16, name=f"wo{j}", tag=f"wo{j}")
                     for j in range(HPC)]
            for j in range(HPC):
                nc.sync.dma_start(out=wo_sb[j], in_=wo_d[j * 128:(j + 1) * 128, :])
            for st in range(NST):
                y_row = y_pool.tile([128, D], F32, name="y_row", tag="y_row")
                # j-major so the stationary operand (outT[j] slice) is reused
                # across the 4 output chunks (1 LDWEIGHTS per 4 matmuls).
                yp_ps = [psD.tile([128, 512], F32, name=f"yp_ps{mc}", tag=f"yp_ps{mc}")
                         for mc in range(NSC)]
                for j in range(HPC):
                    for mc in range(NSC):
                        nc.tensor.matmul(
                            yp_ps[mc],
                            lhsT=(outT[j][:, st * 128:(st + 1) * 128]),
                            rhs=(wo_sb[j][:, mc * 512:(mc + 1) * 512]),
                            start=(j == 0), stop=(j == HPC - 1),
                        )
                for mc in range(NSC):
                    nc.vector.tensor_copy(y_row[:, mc * 512:(mc + 1) * 512], yp_ps[mc])
                nc.scalar.dma_start(out=yp_d[st * 128:(st + 1) * 128, :], in_=y_row)
    return nc


class SpmdRunner:
    """Donation-free SPMD runner: the kernel writes every output element,
    so PJRT-allocated (uninitialized) output buffers are fine and no
    zero-buffer round trip is needed per call."""

    def __init__(self, nc, n_cores: int):
        bass2jax.install_neuronx_cc_hook()
        self.nc = nc
        self.n_cores = n_cores
        partition_name = nc.partition_id_tensor.name if nc.partition_id_tensor else None

        in_names, out_names, out_avals = [], [], []
        for alloc in nc.m.functions[0].allocations:
            if not isinstance(alloc, mybir.MemoryLocationSet):
                continue
            name = alloc.memorylocations[0].name
            if alloc.kind == "ExternalInput":
                if name != partition_name:
                    in_names.append(name)
            elif alloc.kind == "ExternalOutput":
                out_names.append(name)
                shape = tuple(alloc.tensor_shape)
                dtype = mybir.dt.np(alloc.dtype)
                out_avals.append(jax.core.ShapedArray(shape, dtype))
        self.in_names = in_names
        self.out_names = out_names
        self.out_avals = out_avals

        all_in_names = list(in_names)
        if partition_name is not None:
            all_in_names.append(partition_name)

        def _body(*args):
            operands = list(args)
            if partition_name is not None:
                operands.append(partition_id_tensor())
            outs = bass2jax._bass_exec_p.bind(
                *operands,
                out_avals=tuple(out_avals),
                in_names=tuple(all_in_names),
                out_names=tuple(out_names),
                lowering_input_output_aliases=(),
                sim_require_finite=True,
                sim_require_nnan=True,
                nc=nc,
            )
            return tuple(outs)

        devices = jax.devices()[:n_cores]
        self.mesh = Mesh(np.asarray(devices), ("core",))
        in_specs = (PartitionSpec("core"),) * len(in_names)
        out_specs = (PartitionSpec("core"),) * len(out_names)
        self.jitted = jax.jit(
            shard_map(_body, mesh=self.mesh, in_specs=in_specs,
                      out_specs=out_specs, check_rep=False),
            keep_unused=True,
        )
        self.sharding = jax.sharding.NamedSharding(self.mesh, PartitionSpec("core"))

    def stage(self, in_maps):
        assert len(in_maps) == self.n_cores
        arrs = [
            np.concatenate([np.asarray(in_maps[c][n]) for c in range(self.n_cores)], axis=0)
            for n in self.in_names
        ]
        staged = [jax.device_put(a, self.sharding) for a in arrs]
        jax.block_until_ready(staged)
        return staged

    def run_staged(self, staged):
        outs = self.jitted(*staged)
        jax.block_until_ready(outs)
        return outs

    def __call__(self, in_maps):
        staged = self.stage(in_maps)
        outs = self.run_staged(staged)
        res = []
        for c in range(self.n_cores):
            res.append({
                name: np.asarray(outs[i]).reshape(self.n_cores, *self.out_avals[i].shape)[c]
                for i, name in enumerate(self.out_names)
            })
        return res


_NC_CACHE: dict = {}


def _get_runner():
    if "runner" not in _NC_CACHE:
        _NC_CACHE["runner"] = SpmdRunner(_build(), N_CORES)
    return _NC_CACHE["runner"]


def _host_inputs(x, Wq_sem, Wk_sem, Wq_geo, Wk_geo, Wv, Wo):
    # RoPE tables
    inv_freq = 1.0 / (ROPE_BASE ** (np.arange(0, 64, 2, dtype=np.float32) / 64.0))
    t = np.arange(S, dtype=np.float32)
    freqs = np.outer(t, inv_freq)  # [S, 32]
    cosT = np.cos(freqs).T.astype(np.float32)  # [32, S]
    sinT = np.sin(freqs).T.astype(np.float32)
    cs = np.zeros((128, 2 * S), np.float32)
    cs[64:96, :S] = cosT
    cs[96:128, :S] = cosT
    cs[64:96, S:] = sinT
    cs[96:128, S:] = sinT
    cs = cs.astype(NP_BF16)

    # causal mask variants: mask[kl, dj*512 + ql] = ql >= dj*128 + kl
    ql = np.arange(512)
    kl = np.arange(128)
    mask = np.zeros((128, 4 * 512), np.float32)
    for dj in range(4):
        mask[:, dj * 512:(dj + 1) * 512] = (ql[None, :] >= dj * 128 + kl[:, None])
    mask = mask.astype(NP_BF16)

    ones16 = np.ones((128, 128), NP_BF16)
    ones32 = np.ones((128, 128), np.float32)

    in_maps = []
    for c in range(N_CORES):
        b, g = divmod(c, 4)
        blocks_q, blocks_k = [], []
        for j in range(HPC):
            h = g * HPC + j
            r64 = slice(h * 64, (h + 1) * 64)
            blocks_q.append(np.concatenate([Wq_sem[r64], Wq_geo[r64]], axis=0) * SCALE)
            blocks_k.append(np.concatenate([Wk_sem[r64], Wk_geo[r64]], axis=0))
        wqk = np.ascontiguousarray(np.concatenate(blocks_q + blocks_k, axis=0).T)
        hv = slice(g * HPC * DH, (g + 1) * HPC * DH)
        wv = np.ascontiguousarray(Wv[hv].T)
        wo = np.ascontiguousarray(Wo[:, hv].T)
        xT = np.ascontiguousarray(x[b].T)
        in_maps.append({
            "xT": xT.astype(NP_BF16),
            "wqk": wqk.astype(NP_BF16),
            "wv": wv.astype(NP_BF16),
            "wo": wo.astype(NP_BF16),
            "cs": cs,
            "mask": mask,
            "ones16": ones16,
            "ones32": ones32,
        })
    return in_maps


def kernel(x, Wq_sem, Wk_sem, Wq_geo, Wk_geo, Wv, Wo):
    in_maps = _host_inputs(np.asarray(x), np.asarray(Wq_sem), np.asarray(Wk_sem),
                           np.asarray(Wq_geo), np.asarray(Wk_geo),
                           np.asarray(Wv), np.asarray(Wo))
    res = _get_runner()(in_maps)
    y = np.empty((B, S, D), np.float32)
    for b in range(B):
        y[b] = sum(res[b * 4 + g]["yp"] for g in range(4))
    return y
